# revision 12
# baseline (speedup 1.0000x reference)
"""Trainium2 Bass kernel for nn_Attention (b=4, n=2048, d=1024, 16 heads x 64).

Sharding: 8 cores = 4 batches x 2 head-groups (8 heads each).
Per core: qkv projection (transposed layout), scores^T = K @ Q^T per head
(row-tiled pairs, K=64 contraction), exp on ScalarE, AV via lhsT=[V|ones]
(giving av^T and the softmax denominator for free), normalize, proj.
All matmuls in float32r (1 cyc/row, TF32-class precision).

Stage order A (x^T), C (v), then per head-pair B(hp) -> D(hp) so the
PE-heavy projection of the next pair overlaps the ACT-bound exp of the
current one; proj at the end.

Host side: shards inputs, feeds 8 cores via PJRT/axon, sums the two
head-group partials per batch.
"""
import sys

sys.path.insert(0, "/opt/trn_rl_repo")

import numpy as np

import concourse.bass as bass
import concourse.mybir as mybir
import concourse.tile as tile
from concourse import bacc
from concourse.bass import ts, ds
from concourse.masks import make_identity

F32 = mybir.dt.float32
F32R = mybir.dt.float32r
AF = mybir.ActivationFunctionType

SEQ = 2048
DIM = 1024
H = 8  # heads per core
HD = 64
QK = 1024  # q cols (512) ++ k cols (512) per core
VC = 512  # v cols per core
E = 1024  # output dim
KSUB = DIM // 128  # 8
ITILE = 512
NIT = SEQ // ITILE  # 4
NJS = SEQ // 128  # 16
NHP = H // 2  # 4 head-pairs


def build_attention(iters: int = 1):
    nc = bacc.Bacc("TRN2", target_bir_lowering=False, debug=False)
    x = nc.dram_tensor("x", [SEQ, DIM], F32, kind="ExternalInput")
    w_qk = nc.dram_tensor("w_qk", [DIM, QK], F32, kind="ExternalInput")
    w_v = nc.dram_tensor("w_v", [DIM, VC], F32, kind="ExternalInput")
    w_proj = nc.dram_tensor("w_proj", [VC, E], F32, kind="ExternalInput")
    bias = nc.dram_tensor("bias", [E], F32, kind="ExternalInput")
    out = nc.dram_tensor("out", [SEQ, E], F32, kind="ExternalOutput")

    # DRAM views with contraction dim split for SBUF partition layout
    w_qk_r = w_qk.rearrange("(ko p) c -> p ko c", p=128)  # [128, 8, 1024]
    w_v_r = w_v.rearrange("(ko p) c -> p ko c", p=128)  # [128, 8, 512]
    w_proj_r = w_proj.rearrange("(cs p) e -> p cs e", p=128)  # [128, 4, 1024]

    with tile.TileContext(nc) as tc:
        with (
            tc.tile_pool(name="cpool", bufs=1) as cpool,
            tc.tile_pool(name="psum", bufs=2, space="PSUM") as psum,
            tc.tile_pool(name="psum4", bufs=4, space="PSUM") as psum4,
        ):
            pools = (cpool, psum, psum4)
            if iters == 1:
                one_iter(tc, nc, x, w_qk_r, w_v_r, w_proj_r, bias, out, pools)
            else:
                with tc.For_i(0, iters, 1):
                    one_iter(tc, nc, x, w_qk_r, w_v_r, w_proj_r, bias, out, pools)
    nc.compile()
    return nc


def one_iter(tc, nc, x, w_qk_r, w_v_r, w_proj_r, bias, out, pools):
    cpool, psum, psum4 = pools
    ident = cpool.tile([128, 128], F32, tag="ident")
    make_identity(nc, ident[:])

    v_sb = cpool.tile([128, NJS, H * (HD + 1)], F32R, tag="v")  # per head 65 cols
    v_view = v_sb[:].rearrange("p j (h c) -> p j h c", c=HD + 1)
    # fill with ones via broadcast DMA; stage C overwrites the V columns,
    # leaving the per-head ones column (index HD) for the softmax denominator
    ones_dram = nc.inline_tensor(np.ones((NJS, H * (HD + 1)), np.float32), "ones_fill")
    nc.sync.dma_start(
        v_sb[:],
        ones_dram.ap()[None, :, :].to_broadcast((128, NJS, H * (HD + 1))).bitcast(F32R),
    )
    avT = cpool.tile([128, NHP, SEQ], F32R, tag="avT")

    with (
        tc.tile_pool(name="qkring", bufs=2) as qkring,
        tc.tile_pool(name="epool", bufs=3) as epool,
        tc.tile_pool(name="npool", bufs=1) as npool,
    ):
        with (
            tc.tile_pool(name="xTpool", bufs=1) as xTpool,
            tc.tile_pool(name="streampool", bufs=2) as streampool,
        ):
            xT = xTpool.tile([128, KSUB, SEQ], F32R, tag="xT")

            with tc.tile_pool(name="wvpool", bufs=1) as wvpool:
                # ---- Stage A: x^T via PE transpose (fp32, 128x128 tiles) ----
                for ib in range(SEQ // 128):
                    for half in range(2):
                        x_in = streampool.tile([128, DIM // 2], F32, tag="xin")
                        nc.sync.dma_start(
                            x_in[:], x[ts(ib, 128), ts(half, DIM // 2)]
                        )
                        for ksv in range(KSUB // 2):
                            kabs = half * (KSUB // 2) + ksv
                            pt = psum.tile([128, 128], F32, tag="g")
                            nc.tensor.transpose(
                                pt[:, :128], x_in[:, ts(ksv, 128)], ident[:]
                            )
                            nc.vector.tensor_copy(xT[:, kabs, ts(ib, 128)], pt[:, :128])

                # ---- Stage C: v = x @ w_v (natural layout) ----
                w_v_sb = wvpool.tile([128, KSUB, VC], F32R, tag="wv")
                nc.sync.dma_start(w_v_sb[:], w_v_r[:].bitcast(F32R))
                for jt in range(NJS):
                    ps = psum.tile([128, VC], F32, tag="g")
                    for ksv in range(KSUB):
                        nc.tensor.matmul(
                            ps[:],
                            xT[:, ksv, ts(jt, 128)],
                            w_v_sb[:, ksv, :],
                            start=(ksv == 0),
                            stop=(ksv == KSUB - 1),
                        )
                    nc.vector.tensor_copy(
                        v_view[:, jt, :, 0:HD],
                        ps[:].rearrange("p (h c) -> p h c", c=HD),
                    )

            # ---- per head-pair: B(hp) then D(hp) ----
            for hp in range(NHP):
                # B: q^T and k^T for this pair (c-tile hp -> q, hp+4 -> k)
                qTh = qkring.tile([128, SEQ], F32R, tag="qT", name=f"qT{hp}")
                kTh = qkring.tile([128, SEQ], F32R, tag="kT", name=f"kT{hp}")
                for ct, dest in ((hp, qTh), (hp + 4, kTh)):
                    w_t = streampool.tile([128, KSUB, 128], F32R, tag="wqk")
                    nc.sync.dma_start(w_t[:], w_qk_r[:, :, ts(ct, 128)].bitcast(F32R))
                    for it in range(NIT):
                        ps = psum.tile([128, ITILE], F32, tag="g")
                        for ksv in range(KSUB):
                            nc.tensor.matmul(
                                ps[:],
                                w_t[:, ksv, :],
                                xT[:, ksv, ts(it, ITILE)],
                                start=(ksv == 0),
                                stop=(ksv == KSUB - 1),
                            )
                        nc.vector.tensor_copy(dest[:, ts(it, ITILE)], ps[:])

                # D: attention for this pair
                for it in range(NIT):
                    av_ps = [
                        psum4.tile([HD + 1, ITILE], F32, tag="av", name=f"av{h01}")
                        for h01 in range(2)
                    ]
                    for js in range(NJS):
                        for h01 in range(2):
                            sl = slice(h01 * 64, h01 * 64 + 64)
                            sp = psum.tile([128, ITILE], F32, tag="s")
                            nc.tensor.matmul(
                                sp[:],
                                kTh[sl, ts(js, 128)],
                                qTh[sl, ts(it, ITILE)],
                                start=True,
                                stop=True,
                            )
                            e = epool.tile([128, ITILE], F32R, tag="e")
                            nc.scalar.activation(e[:], sp[:], AF.Exp)
                            nc.tensor.matmul(
                                av_ps[h01][:],
                                v_view[:, js, 2 * hp + h01, :],
                                e[:],
                                start=(js == 0),
                                stop=(js == NJS - 1),
                            )
                    for h01 in range(2):
                        h = 2 * hp + h01
                        rc = npool.tile([1, ITILE], F32, tag="rc")
                        nc.vector.reciprocal(rc[:], av_ps[h01][HD : HD + 1, :])
                        rr = npool.tile([64, ITILE], F32, tag="rr")
                        nc.gpsimd.partition_broadcast(rr[:], rc[:])
                        if h01 == 0:
                            nc.vector.tensor_mul(
                                avT[0:64, h // 2, ts(it, ITILE)],
                                av_ps[h01][0:HD, :],
                                rr[:],
                            )
                        else:
                            tmp = npool.tile([64, ITILE], F32R, tag="tmp")
                            nc.vector.tensor_mul(tmp[:], av_ps[h01][0:HD, :], rr[:])
                            nc.sync.dma_start(avT[64:128, h // 2, ts(it, ITILE)], tmp[:])

        # ---- Stage E: out = avRow @ w_proj + bias ----
        with tc.tile_pool(name="wpool", bufs=1) as wpool, tc.tile_pool(
            name="opool", bufs=3
        ) as opool:
            wproj_sb = wpool.tile([128, VC // 128, E], F32R, tag="wproj")
            nc.sync.dma_start(wproj_sb[:], w_proj_r[:].bitcast(F32R))
            bias_rep = wpool.tile([128, E], F32, tag="bias")
            nc.sync.dma_start(bias_rep[:], bias[None, :].to_broadcast((128, E)))
            for it in range(SEQ // 128):
                for et in range(E // ITILE):
                    ps = psum.tile([128, ITILE], F32, tag="s")
                    for cs in range(VC // 128):
                        nc.tensor.matmul(
                            ps[:],
                            avT[:, cs, ts(it, 128)],
                            wproj_sb[:, cs, ts(et, ITILE)],
                            start=(cs == 0),
                            stop=(cs == VC // 128 - 1),
                        )
                    o = opool.tile([128, ITILE], F32, tag="o")
                    nc.vector.tensor_add(o[:], ps[:], bias_rep[:, ts(et, ITILE)])
                    nc.sync.dma_start(out[ts(it, 128), ts(et, ITILE)], o[:])


# ---------------- host side ----------------

_CACHE = {}


def _get_runner():
    if "runner" not in _CACHE:
        import jax
        from jax.sharding import Mesh, PartitionSpec
        from jax.experimental.shard_map import shard_map
        from concourse import bass2jax

        nc = build_attention(iters=1)
        bass2jax.install_neuronx_cc_hook()

        in_names, out_names, out_avals, zero_shapes = [], [], [], []
        partition_name = nc.partition_id_tensor.name if nc.partition_id_tensor else None
        for alloc in nc.m.functions[0].allocations:
            if not isinstance(alloc, mybir.MemoryLocationSet):
                continue
            name = alloc.memorylocations[0].name
            if alloc.kind == "ExternalInput":
                if name != partition_name:
                    in_names.append(name)
            elif alloc.kind == "ExternalOutput":
                out_names.append(name)
                shape = tuple(alloc.tensor_shape)
                dtype = mybir.dt.np(alloc.dtype)
                out_avals.append(jax.core.ShapedArray(shape, dtype))
                zero_shapes.append((shape, dtype))
        n_params = len(in_names)
        n_outs = len(out_avals)
        all_names = in_names + out_names
        if partition_name is not None:
            all_names = all_names + [partition_name]
        donate = tuple(range(n_params, n_params + n_outs))

        def _body(*args):
            operands = list(args)
            if partition_name is not None:
                operands.append(bass2jax.partition_id_tensor())
            outs = bass2jax._bass_exec_p.bind(
                *operands,
                out_avals=tuple(out_avals),
                in_names=tuple(all_names),
                out_names=tuple(out_names),
                lowering_input_output_aliases=(),
                sim_require_finite=True,
                sim_require_nnan=True,
                nc=nc,
            )
            return tuple(outs)

        devices = jax.devices()[:8]
        mesh = Mesh(np.asarray(devices), ("core",))
        in_specs = (PartitionSpec("core"),) * (n_params + n_outs)
        out_specs = (PartitionSpec("core"),) * n_outs
        sharded = jax.jit(
            shard_map(
                _body,
                mesh=mesh,
                in_specs=in_specs,
                out_specs=out_specs,
                check_rep=False,
            ),
            donate_argnums=donate,
            keep_unused=True,
        )
        _CACHE["runner"] = (sharded, in_names, out_names, out_avals, zero_shapes)
    return _CACHE["runner"]


def _shard_inputs(x, w_qkv, w_proj, b_proj):
    """Per-core input dicts. Core c: batch c//2, head-group c%2."""
    SCALE = HD**-0.5
    in_maps = []
    zeros_bias = np.zeros_like(b_proj)
    for c in range(8):
        b = c // 2
        hg = c % 2
        qs = slice(hg * 512, (hg + 1) * 512)
        ks = slice(1024 + hg * 512, 1024 + (hg + 1) * 512)
        vs = slice(2048 + hg * 512, 2048 + (hg + 1) * 512)
        w_qk_c = np.concatenate(
            [w_qkv[:, qs] * np.float32(SCALE), w_qkv[:, ks]], axis=1
        )
        in_maps.append(
            {
                "x": np.ascontiguousarray(x[b]),
                "w_qk": np.ascontiguousarray(w_qk_c),
                "w_v": np.ascontiguousarray(w_qkv[:, vs]),
                "w_proj": np.ascontiguousarray(w_proj[hg * 512 : (hg + 1) * 512]),
                "bias": b_proj if hg == 0 else zeros_bias,
            }
        )
    return in_maps


def kernel(x, w_qkv, w_proj, b_proj):
    import jax
    import jax.numpy as jnp

    x = np.asarray(x, dtype=np.float32)
    w_qkv = np.asarray(w_qkv, dtype=np.float32)
    w_proj = np.asarray(w_proj, dtype=np.float32)
    b_proj = np.asarray(b_proj, dtype=np.float32)

    sharded, in_names, out_names, out_avals, zero_shapes = _get_runner()
    in_maps = _shard_inputs(x, w_qkv, w_proj, b_proj)
    concat_in = [
        np.concatenate([in_maps[c][name] for c in range(8)], axis=0)
        for name in in_names
    ]
    zeros = [jnp.zeros((8 * s[0], *s[1:]), dt) for (s, dt) in zero_shapes]
    outs = sharded(*concat_in, *zeros)
    out_np = np.asarray(outs[out_names.index("out")]).reshape(8, SEQ, E)
    full = np.empty((4, SEQ, E), dtype=np.float32)
    for b in range(4):
        full[b] = out_np[2 * b] + out_np[2 * b + 1]
    return full


# revision 13
# speedup vs baseline: 2.6595x; 2.6595x over previous
"""Trainium2 Bass kernel for nn_Attention (b=4, n=2048, d=1024, 16 heads x 64).

Sharding: 8 cores = 4 batches x 2 head-groups (8 heads each).
Per core: qkv projection (transposed layout), scores^T = K @ Q^T per head
(row-tiled pairs, K=64 contraction), exp on ScalarE, AV via lhsT=[V|ones]
(giving av^T and the softmax denominator for free), normalize, proj.
All matmuls in float32r (1 cyc/row, TF32-class precision).

Stage order A (x^T), C (v), then per head-pair B(hp) -> D(hp) so the
PE-heavy projection of the next pair overlaps the ACT-bound exp of the
current one; proj at the end.

Host side: shards inputs, feeds 8 cores via PJRT/axon, sums the two
head-group partials per batch.
"""
import sys

sys.path.insert(0, "/opt/trn_rl_repo")

import numpy as np

import concourse.bass as bass
import concourse.mybir as mybir
import concourse.tile as tile
from concourse import bacc
from concourse.bass import ts, ds
from concourse.masks import make_identity

F32 = mybir.dt.float32
F32R = mybir.dt.float32r
AF = mybir.ActivationFunctionType

SEQ = 2048
DIM = 1024
H = 8  # heads per core
HD = 64
QK = 1024  # q cols (512) ++ k cols (512) per core
VC = 512  # v cols per core
E = 1024  # output dim
KSUB = DIM // 128  # 8
ITILE = 512
NIT = SEQ // ITILE  # 4
NJS = SEQ // 128  # 16
NHP = H // 2  # 4 head-pairs


def build_attention(iters: int = 1, stages: int = 5):
    nc = bacc.Bacc("TRN2", target_bir_lowering=False, debug=False)
    x = nc.dram_tensor("x", [SEQ, DIM], F32, kind="ExternalInput")
    w_qk = nc.dram_tensor("w_qk", [DIM, QK], F32, kind="ExternalInput")
    w_v = nc.dram_tensor("w_v", [DIM, VC], F32, kind="ExternalInput")
    w_proj = nc.dram_tensor("w_proj", [VC, E], F32, kind="ExternalInput")
    bias = nc.dram_tensor("bias", [E], F32, kind="ExternalInput")
    out = nc.dram_tensor("out", [SEQ, E], F32, kind="ExternalOutput")

    # DRAM views with contraction dim split for SBUF partition layout
    w_qk_r = w_qk.rearrange("(ko p) c -> p ko c", p=128)  # [128, 8, 1024]
    w_v_r = w_v.rearrange("(ko p) c -> p ko c", p=128)  # [128, 8, 512]
    w_proj_r = w_proj.rearrange("(cs p) e -> p cs e", p=128)  # [128, 4, 1024]

    with tile.TileContext(nc) as tc:
        with (
            tc.tile_pool(name="cpool", bufs=1) as cpool,
            tc.tile_pool(name="psum", bufs=2, space="PSUM") as psum,
            tc.tile_pool(name="psum4", bufs=4, space="PSUM") as psum4,
        ):
            pools = (cpool, psum, psum4)
            if iters == 1:
                one_iter(tc, nc, x, w_qk_r, w_v_r, w_proj_r, bias, out, pools, stages)
            else:
                with tc.For_i(0, iters, 1):
                    one_iter(tc, nc, x, w_qk_r, w_v_r, w_proj_r, bias, out, pools, stages)
    nc.compile()
    return nc


def one_iter(tc, nc, x, w_qk_r, w_v_r, w_proj_r, bias, out, pools, stages=5):
    cpool, psum, psum4 = pools
    ident = cpool.tile([128, 128], F32, tag="ident")
    make_identity(nc, ident[:])

    v_sb = cpool.tile([128, NJS, H * (HD + 1)], F32R, tag="v")  # per head 65 cols
    v_view = v_sb[:].rearrange("p j (h c) -> p j h c", c=HD + 1)
    # fill with ones via broadcast DMA; stage C overwrites the V columns,
    # leaving the per-head ones column (index HD) for the softmax denominator
    ones_dram = nc.inline_tensor(np.ones((NJS, H * (HD + 1)), np.float32), "ones_fill")
    nc.sync.dma_start(
        v_sb[:],
        ones_dram.ap()[None, :, :].to_broadcast((128, NJS, H * (HD + 1))).bitcast(F32R),
    )
    avT = cpool.tile([128, NHP, SEQ], F32R, tag="avT")

    with (
        tc.tile_pool(name="qkring", bufs=2) as qkring,
        tc.tile_pool(name="epool", bufs=3) as epool,
        tc.tile_pool(name="npool", bufs=1) as npool,
    ):
        with (
            tc.tile_pool(name="xTpool", bufs=1) as xTpool,
            tc.tile_pool(name="streampool", bufs=2) as streampool,
        ):
            xT = xTpool.tile([128, KSUB, SEQ], F32R, tag="xT")

            with tc.tile_pool(name="wvpool", bufs=1) as wvpool:
                # ---- Stage A: x^T via PE transpose (fp32, 128x128 tiles) ----
                for ib in range(SEQ // 128):
                    for half in range(2):
                        x_in = streampool.tile([128, DIM // 2], F32, tag="xin")
                        nc.sync.dma_start(
                            x_in[:], x[ts(ib, 128), ts(half, DIM // 2)]
                        )
                        for ksv in range(KSUB // 2):
                            kabs = half * (KSUB // 2) + ksv
                            pt = psum.tile([128, 128], F32, tag="g")
                            nc.tensor.transpose(
                                pt[:, :128], x_in[:, ts(ksv, 128)], ident[:]
                            )
                            nc.vector.tensor_copy(xT[:, kabs, ts(ib, 128)], pt[:, :128])

                if stages <= 1:
                    nc.sync.dma_start(
                        out[:].rearrange("(a p) e -> p (a e)", p=128),
                        xT[:].rearrange("p k s -> p (k s)").bitcast(F32),
                    )
                    return
                # ---- Stage C: v = x @ w_v (natural layout) ----
                w_v_sb = wvpool.tile([128, KSUB, VC], F32R, tag="wv")
                nc.sync.dma_start(w_v_sb[:], w_v_r[:].bitcast(F32R))
                for jt in range(NJS):
                    ps = psum.tile([128, VC], F32, tag="g")
                    for ksv in range(KSUB):
                        nc.tensor.matmul(
                            ps[:],
                            xT[:, ksv, ts(jt, 128)],
                            w_v_sb[:, ksv, :],
                            start=(ksv == 0),
                            stop=(ksv == KSUB - 1),
                        )
                    nc.vector.tensor_copy(
                        v_view[:, jt, :, 0:HD],
                        ps[:].rearrange("p (h c) -> p h c", c=HD),
                    )

            if stages <= 2:
                nc.sync.dma_start(
                    out[0:1024, :].rearrange("(a p) e -> p (a e)", p=128),
                    v_sb[:].rearrange("p j c -> p (j c)").bitcast(F32),
                )
                return
            # ---- per head-pair: B(hp) then D(hp) ----
            for hp in range(NHP):
                # B: q^T and k^T for this pair (c-tile hp -> q, hp+4 -> k)
                qTh = qkring.tile([128, SEQ], F32R, tag="qT", name=f"qT{hp}")
                kTh = qkring.tile([128, SEQ], F32R, tag="kT", name=f"kT{hp}")
                for ct, dest in ((hp, qTh), (hp + 4, kTh)):
                    w_t = streampool.tile([128, KSUB, 128], F32R, tag="wqk")
                    nc.sync.dma_start(w_t[:], w_qk_r[:, :, ts(ct, 128)].bitcast(F32R))
                    for it in range(NIT):
                        ps = psum.tile([128, ITILE], F32, tag="g")
                        for ksv in range(KSUB):
                            nc.tensor.matmul(
                                ps[:],
                                w_t[:, ksv, :],
                                xT[:, ksv, ts(it, ITILE)],
                                start=(ksv == 0),
                                stop=(ksv == KSUB - 1),
                            )
                        nc.vector.tensor_copy(dest[:, ts(it, ITILE)], ps[:])

                if stages <= 3:
                    nc.sync.dma_start(
                        out[ds(hp * 256, 128), :].rearrange("(a p) e -> p (a e)", p=128),
                        qTh[:].bitcast(F32),
                    )
                    nc.sync.dma_start(
                        out[ds(hp * 256 + 128, 128), :].rearrange("(a p) e -> p (a e)", p=128),
                        kTh[:].bitcast(F32),
                    )
                    continue
                # D: attention for this pair
                for it in range(NIT):
                    av_ps = [
                        psum4.tile([HD + 1, ITILE], F32, tag="av", name=f"av{h01}")
                        for h01 in range(2)
                    ]
                    for js in range(NJS):
                        for h01 in range(2):
                            sl = slice(h01 * 64, h01 * 64 + 64)
                            sp = psum.tile([128, ITILE], F32, tag="s")
                            nc.tensor.matmul(
                                sp[:],
                                kTh[sl, ts(js, 128)],
                                qTh[sl, ts(it, ITILE)],
                                start=True,
                                stop=True,
                            )
                            e = epool.tile([128, ITILE], F32R, tag="e")
                            nc.scalar.activation(e[:], sp[:], AF.Exp)
                            nc.tensor.matmul(
                                av_ps[h01][:],
                                v_view[:, js, 2 * hp + h01, :],
                                e[:],
                                start=(js == 0),
                                stop=(js == NJS - 1),
                            )
                    for h01 in range(2):
                        h = 2 * hp + h01
                        rc = npool.tile([1, ITILE], F32, tag="rc")
                        nc.vector.reciprocal(rc[:], av_ps[h01][HD : HD + 1, :])
                        rr = npool.tile([64, ITILE], F32, tag="rr")
                        nc.gpsimd.partition_broadcast(rr[:], rc[:])
                        if h01 == 0:
                            nc.vector.tensor_mul(
                                avT[0:64, h // 2, ts(it, ITILE)],
                                av_ps[h01][0:HD, :],
                                rr[:],
                            )
                        else:
                            tmp = npool.tile([64, ITILE], F32R, tag="tmp")
                            nc.vector.tensor_mul(tmp[:], av_ps[h01][0:HD, :], rr[:])
                            nc.sync.dma_start(avT[64:128, h // 2, ts(it, ITILE)], tmp[:])

        if stages <= 4:
            nc.sync.dma_start(
                out[0:1024, :].rearrange("(a p) e -> p (a e)", p=128),
                avT[:].rearrange("p k s -> p (k s)").bitcast(F32),
            )
            return
        # ---- Stage E: out = avRow @ w_proj + bias ----
        with tc.tile_pool(name="wpool", bufs=1) as wpool, tc.tile_pool(
            name="opool", bufs=3
        ) as opool:
            wproj_sb = wpool.tile([128, VC // 128, E], F32R, tag="wproj")
            nc.sync.dma_start(wproj_sb[:], w_proj_r[:].bitcast(F32R))
            bias_rep = wpool.tile([128, E], F32, tag="bias")
            nc.sync.dma_start(bias_rep[:], bias[None, :].to_broadcast((128, E)))
            for it in range(SEQ // 128):
                for et in range(E // ITILE):
                    ps = psum.tile([128, ITILE], F32, tag="s")
                    for cs in range(VC // 128):
                        nc.tensor.matmul(
                            ps[:],
                            avT[:, cs, ts(it, 128)],
                            wproj_sb[:, cs, ts(et, ITILE)],
                            start=(cs == 0),
                            stop=(cs == VC // 128 - 1),
                        )
                    o = opool.tile([128, ITILE], F32, tag="o")
                    nc.vector.tensor_add(o[:], ps[:], bias_rep[:, ts(et, ITILE)])
                    nc.sync.dma_start(out[ts(it, 128), ts(et, ITILE)], o[:])


# ---------------- host side ----------------

_CACHE = {}


def _get_runner():
    if "runner" not in _CACHE:
        import jax
        from jax.sharding import Mesh, PartitionSpec
        from jax.experimental.shard_map import shard_map
        from concourse import bass2jax

        nc = build_attention(iters=1)
        bass2jax.install_neuronx_cc_hook()

        in_names, out_names, out_avals, zero_shapes = [], [], [], []
        partition_name = nc.partition_id_tensor.name if nc.partition_id_tensor else None
        for alloc in nc.m.functions[0].allocations:
            if not isinstance(alloc, mybir.MemoryLocationSet):
                continue
            name = alloc.memorylocations[0].name
            if alloc.kind == "ExternalInput":
                if name != partition_name:
                    in_names.append(name)
            elif alloc.kind == "ExternalOutput":
                out_names.append(name)
                shape = tuple(alloc.tensor_shape)
                dtype = mybir.dt.np(alloc.dtype)
                out_avals.append(jax.core.ShapedArray(shape, dtype))
                zero_shapes.append((shape, dtype))
        n_params = len(in_names)
        n_outs = len(out_avals)
        all_names = in_names + out_names
        if partition_name is not None:
            all_names = all_names + [partition_name]
        donate = tuple(range(n_params, n_params + n_outs))

        def _body(*args):
            operands = list(args)
            if partition_name is not None:
                operands.append(bass2jax.partition_id_tensor())
            outs = bass2jax._bass_exec_p.bind(
                *operands,
                out_avals=tuple(out_avals),
                in_names=tuple(all_names),
                out_names=tuple(out_names),
                lowering_input_output_aliases=(),
                sim_require_finite=True,
                sim_require_nnan=True,
                nc=nc,
            )
            return tuple(outs)

        devices = jax.devices()[:8]
        mesh = Mesh(np.asarray(devices), ("core",))
        in_specs = (PartitionSpec("core"),) * (n_params + n_outs)
        out_specs = (PartitionSpec("core"),) * n_outs
        sharded = jax.jit(
            shard_map(
                _body,
                mesh=mesh,
                in_specs=in_specs,
                out_specs=out_specs,
                check_rep=False,
            ),
            donate_argnums=donate,
            keep_unused=True,
        )
        _CACHE["runner"] = (sharded, in_names, out_names, out_avals, zero_shapes)
    return _CACHE["runner"]


def _shard_inputs(x, w_qkv, w_proj, b_proj):
    """Per-core input dicts. Core c: batch c//2, head-group c%2."""
    SCALE = HD**-0.5
    in_maps = []
    zeros_bias = np.zeros_like(b_proj)
    for c in range(8):
        b = c // 2
        hg = c % 2
        qs = slice(hg * 512, (hg + 1) * 512)
        ks = slice(1024 + hg * 512, 1024 + (hg + 1) * 512)
        vs = slice(2048 + hg * 512, 2048 + (hg + 1) * 512)
        w_qk_c = np.concatenate(
            [w_qkv[:, qs] * np.float32(SCALE), w_qkv[:, ks]], axis=1
        )
        in_maps.append(
            {
                "x": np.ascontiguousarray(x[b]),
                "w_qk": np.ascontiguousarray(w_qk_c),
                "w_v": np.ascontiguousarray(w_qkv[:, vs]),
                "w_proj": np.ascontiguousarray(w_proj[hg * 512 : (hg + 1) * 512]),
                "bias": b_proj if hg == 0 else zeros_bias,
            }
        )
    return in_maps


def kernel(x, w_qkv, w_proj, b_proj):
    import jax
    import jax.numpy as jnp

    x = np.asarray(x, dtype=np.float32)
    w_qkv = np.asarray(w_qkv, dtype=np.float32)
    w_proj = np.asarray(w_proj, dtype=np.float32)
    b_proj = np.asarray(b_proj, dtype=np.float32)

    sharded, in_names, out_names, out_avals, zero_shapes = _get_runner()
    in_maps = _shard_inputs(x, w_qkv, w_proj, b_proj)
    concat_in = [
        np.concatenate([in_maps[c][name] for c in range(8)], axis=0)
        for name in in_names
    ]
    zeros = [jnp.zeros((8 * s[0], *s[1:]), dt) for (s, dt) in zero_shapes]
    outs = sharded(*concat_in, *zeros)
    out_np = np.asarray(outs[out_names.index("out")]).reshape(8, SEQ, E)
    full = np.empty((4, SEQ, E), dtype=np.float32)
    for b in range(4):
        full[b] = out_np[2 * b] + out_np[2 * b + 1]
    return full


# revision 14
# speedup vs baseline: 2.8412x; 1.0683x over previous
"""Trainium2 Bass kernel for nn_Attention (b=4, n=2048, d=1024, 16 heads x 64).

Sharding: 8 cores = 4 batches x 2 head-groups (8 heads each).
Per core: qkv projection (transposed layout), scores^T = K @ Q^T per head
(row-tiled pairs, K=64 contraction), exp on ScalarE, AV via lhsT=[V|ones]
(giving av^T and the softmax denominator for free), normalize, proj.
All matmuls in float32r (1 cyc/row, TF32-class precision).

Stage order A (x^T), C (v), then per head-pair B(hp) -> D(hp) so the
PE-heavy projection of the next pair overlaps the ACT-bound exp of the
current one; proj at the end.

Host side: shards inputs, feeds 8 cores via PJRT/axon, sums the two
head-group partials per batch.
"""
import sys

sys.path.insert(0, "/opt/trn_rl_repo")

import numpy as np

import concourse.bass as bass
import concourse.mybir as mybir
import concourse.tile as tile
from concourse import bacc
from concourse.bass import ts, ds
from concourse.masks import make_identity

F32 = mybir.dt.float32
F32R = mybir.dt.float32r
AF = mybir.ActivationFunctionType

SEQ = 2048
DIM = 1024
H = 8  # heads per core
HD = 64
QK = 1024  # q cols (512) ++ k cols (512) per core
VC = 512  # v cols per core
E = 1024  # output dim
KSUB = DIM // 128  # 8
ITILE = 512
NIT = SEQ // ITILE  # 4
NJS = SEQ // 128  # 16
NHP = H // 2  # 4 head-pairs


def build_attention(iters: int = 1, stages: int = 5):
    nc = bacc.Bacc("TRN2", target_bir_lowering=False, debug=False)
    x = nc.dram_tensor("x", [SEQ, DIM], F32, kind="ExternalInput")
    w_qk = nc.dram_tensor("w_qk", [DIM, QK], F32, kind="ExternalInput")
    w_v = nc.dram_tensor("w_v", [DIM, VC], F32, kind="ExternalInput")
    w_proj = nc.dram_tensor("w_proj", [VC, E], F32, kind="ExternalInput")
    bias = nc.dram_tensor("bias", [E], F32, kind="ExternalInput")
    out = nc.dram_tensor("out", [SEQ, E], F32, kind="ExternalOutput")

    # DRAM views with contraction dim split for SBUF partition layout
    w_qk_r = w_qk.rearrange("(ko p) c -> p ko c", p=128)  # [128, 8, 1024]
    w_v_r = w_v.rearrange("(ko p) c -> p ko c", p=128)  # [128, 8, 512]
    w_proj_r = w_proj.rearrange("(cs p) e -> p cs e", p=128)  # [128, 4, 1024]

    with tile.TileContext(nc) as tc:
        with (
            tc.tile_pool(name="cpool", bufs=1) as cpool,
            tc.tile_pool(name="psum", bufs=2, space="PSUM") as psum,
            tc.tile_pool(name="psum4", bufs=4, space="PSUM") as psum4,
        ):
            pools = (cpool, psum, psum4)
            if iters == 1:
                one_iter(tc, nc, x, w_qk_r, w_v_r, w_proj_r, bias, out, pools, stages)
            else:
                with tc.For_i(0, iters, 1):
                    one_iter(tc, nc, x, w_qk_r, w_v_r, w_proj_r, bias, out, pools, stages)
    nc.compile()
    return nc


def one_iter(tc, nc, x, w_qk_r, w_v_r, w_proj_r, bias, out, pools, stages=5):
    cpool, psum, psum4 = pools
    ident = cpool.tile([128, 128], F32, tag="ident")
    make_identity(nc, ident[:])

    v_sb = cpool.tile([128, NJS, H * (HD + 1)], F32R, tag="v")  # per head 65 cols
    v_view = v_sb[:].rearrange("p j (h c) -> p j h c", c=HD + 1)
    # fill with ones via broadcast DMA; stage C overwrites the V columns,
    # leaving the per-head ones column (index HD) for the softmax denominator
    ones_dram = nc.inline_tensor(np.ones((NJS, H * (HD + 1)), np.float32), "ones_fill")
    nc.sync.dma_start(
        v_sb[:],
        ones_dram.ap()[None, :, :].to_broadcast((128, NJS, H * (HD + 1))).bitcast(F32R),
    )
    avT = cpool.tile([128, NHP, SEQ], F32R, tag="avT")

    with (
        tc.tile_pool(name="qkring", bufs=2) as qkring,
        tc.tile_pool(name="epool", bufs=3) as epool,
        tc.tile_pool(name="npool", bufs=1) as npool,
    ):
        with (
            tc.tile_pool(name="xTpool", bufs=1) as xTpool,
            tc.tile_pool(name="streampool", bufs=2) as streampool,
        ):
            xT = xTpool.tile([128, KSUB, SEQ], F32R, tag="xT")

            with tc.tile_pool(name="wvpool", bufs=1) as wvpool:
                # ---- Stage A: x^T via PE transpose (fp32, 128x128 tiles) ----
                for ib in range(SEQ // 128):
                    for half in range(2):
                        x_in = streampool.tile([128, DIM // 2], F32, tag="xin")
                        nc.sync.dma_start(
                            x_in[:], x[ts(ib, 128), ts(half, DIM // 2)]
                        )
                        for ksv in range(KSUB // 2):
                            kabs = half * (KSUB // 2) + ksv
                            pt = psum.tile([128, 128], F32, tag="g")
                            nc.tensor.transpose(
                                pt[:, :128], x_in[:, ts(ksv, 128)], ident[:]
                            )
                            nc.vector.tensor_copy(xT[:, kabs, ts(ib, 128)], pt[:, :128])

                if stages <= 1:
                    out_r = out.rearrange("(p a) e -> p (a e)", p=128)
                    nc.sync.dma_start(
                        out_r[:, :], xT[:].rearrange("p k s -> p (k s)").bitcast(F32)
                    )
                    return
                # ---- Stage C: v = x @ w_v (natural layout) ----
                w_v_sb = wvpool.tile([128, KSUB, VC], F32R, tag="wv")
                nc.sync.dma_start(w_v_sb[:], w_v_r[:].bitcast(F32R))
                for jt in range(NJS):
                    ps = psum.tile([128, VC], F32, tag="g")
                    for ksv in range(KSUB):
                        nc.tensor.matmul(
                            ps[:],
                            xT[:, ksv, ts(jt, 128)],
                            w_v_sb[:, ksv, :],
                            start=(ksv == 0),
                            stop=(ksv == KSUB - 1),
                        )
                    nc.vector.tensor_copy(
                        v_view[:, jt, :, 0:HD],
                        ps[:].rearrange("p (h c) -> p h c", c=HD),
                    )

            if stages <= 2:
                out_r = out.rearrange("(p a) e -> p (a e)", p=128)
                nc.sync.dma_start(
                    out_r[:, 0 : NJS * H * (HD + 1)],
                    v_sb[:].rearrange("p j c -> p (j c)").bitcast(F32),
                )
                return
            # ---- per head-pair: B(hp) then D(hp) ----
            for hp in range(NHP):
                # B: q^T and k^T for this pair (c-tile hp -> q, hp+4 -> k)
                qTh = qkring.tile([128, SEQ], F32R, tag="qT", name=f"qT{hp}")
                kTh = qkring.tile([128, SEQ], F32R, tag="kT", name=f"kT{hp}")
                for ct, dest in ((hp, qTh), (hp + 4, kTh)):
                    w_t = streampool.tile([128, KSUB, 128], F32R, tag="wqk")
                    nc.sync.dma_start(w_t[:], w_qk_r[:, :, ts(ct, 128)].bitcast(F32R))
                    for it in range(NIT):
                        ps = psum.tile([128, ITILE], F32, tag="g")
                        for ksv in range(KSUB):
                            nc.tensor.matmul(
                                ps[:],
                                w_t[:, ksv, :],
                                xT[:, ksv, ts(it, ITILE)],
                                start=(ksv == 0),
                                stop=(ksv == KSUB - 1),
                            )
                        nc.vector.tensor_copy(dest[:, ts(it, ITILE)], ps[:])

                if stages <= 3:
                    out_r = out.rearrange("(p a) e -> p (a e)", p=128)
                    nc.sync.dma_start(
                        out_r[:, ds(hp * 4096, SEQ)], qTh[:].bitcast(F32)
                    )
                    nc.sync.dma_start(
                        out_r[:, ds(hp * 4096 + SEQ, SEQ)], kTh[:].bitcast(F32)
                    )
                    continue
                # D: attention for this pair
                for it in range(NIT):
                    av_ps = [
                        psum4.tile([HD + 1, ITILE], F32, tag="av", name=f"av{h01}")
                        for h01 in range(2)
                    ]
                    for js in range(NJS):
                        for h01 in range(2):
                            sl = slice(h01 * 64, h01 * 64 + 64)
                            sp = psum.tile([128, ITILE], F32, tag="s")
                            nc.tensor.matmul(
                                sp[:],
                                kTh[sl, ts(js, 128)],
                                qTh[sl, ts(it, ITILE)],
                                start=True,
                                stop=True,
                            )
                            e = epool.tile([128, ITILE], F32R, tag="e")
                            nc.scalar.activation(e[:], sp[:], AF.Exp)
                            nc.tensor.matmul(
                                av_ps[h01][:],
                                v_view[:, js, 2 * hp + h01, :],
                                e[:],
                                start=(js == 0),
                                stop=(js == NJS - 1),
                            )
                    for h01 in range(2):
                        h = 2 * hp + h01
                        rc = npool.tile([1, ITILE], F32, tag="rc")
                        nc.vector.reciprocal(rc[:], av_ps[h01][HD : HD + 1, :])
                        rr = npool.tile([64, ITILE], F32, tag="rr")
                        nc.gpsimd.partition_broadcast(rr[:], rc[:])
                        if h01 == 0:
                            nc.vector.tensor_mul(
                                avT[0:64, h // 2, ts(it, ITILE)],
                                av_ps[h01][0:HD, :],
                                rr[:],
                            )
                        else:
                            tmp = npool.tile([64, ITILE], F32R, tag="tmp")
                            nc.vector.tensor_mul(tmp[:], av_ps[h01][0:HD, :], rr[:])
                            nc.sync.dma_start(avT[64:128, h // 2, ts(it, ITILE)], tmp[:])

        if stages <= 4:
            out_r = out.rearrange("(p a) e -> p (a e)", p=128)
            nc.sync.dma_start(
                out_r[:, 0 : NHP * SEQ],
                avT[:].rearrange("p k s -> p (k s)").bitcast(F32),
            )
            return
        # ---- Stage E: out = avRow @ w_proj + bias ----
        with tc.tile_pool(name="wpool", bufs=1) as wpool, tc.tile_pool(
            name="opool", bufs=3
        ) as opool:
            wproj_sb = wpool.tile([128, VC // 128, E], F32R, tag="wproj")
            nc.sync.dma_start(wproj_sb[:], w_proj_r[:].bitcast(F32R))
            bias_rep = wpool.tile([128, E], F32, tag="bias")
            nc.sync.dma_start(bias_rep[:], bias[None, :].to_broadcast((128, E)))
            for it in range(SEQ // 128):
                for et in range(E // ITILE):
                    ps = psum.tile([128, ITILE], F32, tag="s")
                    for cs in range(VC // 128):
                        nc.tensor.matmul(
                            ps[:],
                            avT[:, cs, ts(it, 128)],
                            wproj_sb[:, cs, ts(et, ITILE)],
                            start=(cs == 0),
                            stop=(cs == VC // 128 - 1),
                        )
                    o = opool.tile([128, ITILE], F32, tag="o")
                    nc.vector.tensor_add(o[:], ps[:], bias_rep[:, ts(et, ITILE)])
                    nc.sync.dma_start(out[ts(it, 128), ts(et, ITILE)], o[:])


# ---------------- host side ----------------

_CACHE = {}


def _get_runner():
    if "runner" not in _CACHE:
        import jax
        from jax.sharding import Mesh, PartitionSpec
        from jax.experimental.shard_map import shard_map
        from concourse import bass2jax

        nc = build_attention(iters=1)
        bass2jax.install_neuronx_cc_hook()

        in_names, out_names, out_avals, zero_shapes = [], [], [], []
        partition_name = nc.partition_id_tensor.name if nc.partition_id_tensor else None
        for alloc in nc.m.functions[0].allocations:
            if not isinstance(alloc, mybir.MemoryLocationSet):
                continue
            name = alloc.memorylocations[0].name
            if alloc.kind == "ExternalInput":
                if name != partition_name:
                    in_names.append(name)
            elif alloc.kind == "ExternalOutput":
                out_names.append(name)
                shape = tuple(alloc.tensor_shape)
                dtype = mybir.dt.np(alloc.dtype)
                out_avals.append(jax.core.ShapedArray(shape, dtype))
                zero_shapes.append((shape, dtype))
        n_params = len(in_names)
        n_outs = len(out_avals)
        all_names = in_names + out_names
        if partition_name is not None:
            all_names = all_names + [partition_name]
        donate = tuple(range(n_params, n_params + n_outs))

        def _body(*args):
            operands = list(args)
            if partition_name is not None:
                operands.append(bass2jax.partition_id_tensor())
            outs = bass2jax._bass_exec_p.bind(
                *operands,
                out_avals=tuple(out_avals),
                in_names=tuple(all_names),
                out_names=tuple(out_names),
                lowering_input_output_aliases=(),
                sim_require_finite=True,
                sim_require_nnan=True,
                nc=nc,
            )
            return tuple(outs)

        devices = jax.devices()[:8]
        mesh = Mesh(np.asarray(devices), ("core",))
        in_specs = (PartitionSpec("core"),) * (n_params + n_outs)
        out_specs = (PartitionSpec("core"),) * n_outs
        sharded = jax.jit(
            shard_map(
                _body,
                mesh=mesh,
                in_specs=in_specs,
                out_specs=out_specs,
                check_rep=False,
            ),
            donate_argnums=donate,
            keep_unused=True,
        )
        _CACHE["runner"] = (sharded, in_names, out_names, out_avals, zero_shapes)
    return _CACHE["runner"]


def _shard_inputs(x, w_qkv, w_proj, b_proj):
    """Per-core input dicts. Core c: batch c//2, head-group c%2."""
    SCALE = HD**-0.5
    in_maps = []
    zeros_bias = np.zeros_like(b_proj)
    for c in range(8):
        b = c // 2
        hg = c % 2
        qs = slice(hg * 512, (hg + 1) * 512)
        ks = slice(1024 + hg * 512, 1024 + (hg + 1) * 512)
        vs = slice(2048 + hg * 512, 2048 + (hg + 1) * 512)
        w_qk_c = np.concatenate(
            [w_qkv[:, qs] * np.float32(SCALE), w_qkv[:, ks]], axis=1
        )
        in_maps.append(
            {
                "x": np.ascontiguousarray(x[b]),
                "w_qk": np.ascontiguousarray(w_qk_c),
                "w_v": np.ascontiguousarray(w_qkv[:, vs]),
                "w_proj": np.ascontiguousarray(w_proj[hg * 512 : (hg + 1) * 512]),
                "bias": b_proj if hg == 0 else zeros_bias,
            }
        )
    return in_maps


def kernel(x, w_qkv, w_proj, b_proj):
    import jax
    import jax.numpy as jnp

    x = np.asarray(x, dtype=np.float32)
    w_qkv = np.asarray(w_qkv, dtype=np.float32)
    w_proj = np.asarray(w_proj, dtype=np.float32)
    b_proj = np.asarray(b_proj, dtype=np.float32)

    sharded, in_names, out_names, out_avals, zero_shapes = _get_runner()
    in_maps = _shard_inputs(x, w_qkv, w_proj, b_proj)
    concat_in = [
        np.concatenate([in_maps[c][name] for c in range(8)], axis=0)
        for name in in_names
    ]
    zeros = [jnp.zeros((8 * s[0], *s[1:]), dt) for (s, dt) in zero_shapes]
    outs = sharded(*concat_in, *zeros)
    out_np = np.asarray(outs[out_names.index("out")]).reshape(8, SEQ, E)
    full = np.empty((4, SEQ, E), dtype=np.float32)
    for b in range(4):
        full[b] = out_np[2 * b] + out_np[2 * b + 1]
    return full


# revision 15
# speedup vs baseline: 3.5578x; 1.2522x over previous
"""Trainium2 Bass kernel for nn_Attention (b=4, n=2048, d=1024, 16 heads x 64).

Sharding: 8 cores = 4 batches x 2 head-groups (8 heads each).

Per core (transposed-layout pipeline, no intermediate transposes):
  A: x^T via XBAR DMA-transpose (bf16)
  B: q^T/k^T = (x @ w_qk)^T per head-pair (bf16 matmuls, fp32 psum)
  C: v = x @ w_v with a ones column appended per head
  D: scores^T = K @ Q^T (row-tiled K=64 pairs) -> exp (ACT, fp32->bf16)
     -> av^T = [V|1]^T @ exp^T, giving the softmax denominator for free;
     normalize with DVE reciprocal + gpsimd partition broadcast
  E: out = av @ w_proj + bias in float32r (accuracy-critical last layer)

dtype choices are empirical: bf16 matmul ~213ns/512-col vs ~1us for f32r;
ACT exp fp32->bf16 runs at full rate (396ns) vs 2.5us for fp32->fp32.
bf16 noise in scores/attention is suppressed by softmax normalization
(common mode) and diffuse averaging over 2048 keys; the final projection
stays f32r because its error passes straight through.

Host side: shards inputs (bf16 casts, q-scale folded into w_q), feeds 8
cores via PJRT/axon, sums the two head-group partials per batch.
"""
import sys

sys.path.insert(0, "/opt/trn_rl_repo")

import ml_dtypes
import numpy as np

import concourse.bass as bass
import concourse.mybir as mybir
import concourse.tile as tile
from concourse import bacc
from concourse.bass import ts, ds

F32 = mybir.dt.float32
F32R = mybir.dt.float32r
BF16 = mybir.dt.bfloat16
AF = mybir.ActivationFunctionType

SEQ = 2048
DIM = 1024
H = 8  # heads per core
HD = 64
QK = 1024  # q cols (512) ++ k cols (512) per core
VC = 512  # v cols per core
E = 1024  # output dim
KSUB = DIM // 128  # 8
ITILE = 512
NIT = SEQ // ITILE  # 4
NJS = SEQ // 128  # 16
NHP = H // 2  # 4 head-pairs


def build_attention(iters: int = 1, stages: int = 5):
    nc = bacc.Bacc("TRN2", target_bir_lowering=False, debug=False)
    x = nc.dram_tensor("x", [SEQ, DIM], BF16, kind="ExternalInput")
    w_qk = nc.dram_tensor("w_qk", [DIM, QK], BF16, kind="ExternalInput")
    w_v = nc.dram_tensor("w_v", [DIM, VC], BF16, kind="ExternalInput")
    w_proj = nc.dram_tensor("w_proj", [VC, E], F32, kind="ExternalInput")
    bias = nc.dram_tensor("bias", [E], F32, kind="ExternalInput")
    out = nc.dram_tensor("out", [SEQ, E], F32, kind="ExternalOutput")

    w_qk_r = w_qk.rearrange("(ko p) c -> p ko c", p=128)  # [128, 8, 1024]
    w_v_r = w_v.rearrange("(ko p) c -> p ko c", p=128)  # [128, 8, 512]
    w_proj_r = w_proj.rearrange("(cs p) e -> p cs e", p=128)  # [128, 4, 1024]

    with tile.TileContext(nc) as tc:
        with (
            tc.tile_pool(name="cpool", bufs=1) as cpool,
            tc.tile_pool(name="qkring", bufs=2) as qkring,
            tc.tile_pool(name="stream", bufs=2) as stream,
            tc.tile_pool(name="epool", bufs=6) as epool,
            tc.tile_pool(name="npool", bufs=2) as npool,
            tc.tile_pool(name="opool", bufs=3) as opool,
            tc.tile_pool(name="psum", bufs=2, space="PSUM") as psum,
            tc.tile_pool(name="psum4", bufs=4, space="PSUM") as psum4,
        ):
            pools = (cpool, qkring, stream, epool, npool, opool, psum, psum4)
            if iters == 1:
                one_iter(tc, nc, x, w_qk_r, w_v_r, w_proj_r, bias, out, pools, stages)
            else:
                with tc.For_i(0, iters, 1):
                    one_iter(
                        tc, nc, x, w_qk_r, w_v_r, w_proj_r, bias, out, pools, stages
                    )
    nc.compile()
    return nc


def one_iter(tc, nc, x, w_qk_r, w_v_r, w_proj_r, bias, out, pools, stages=5):
    cpool, qkring, stream, epool, npool, opool, psum, psum4 = pools

    v_sb = cpool.tile([128, NJS, H * (HD + 1)], BF16, tag="v")  # per head 65 cols
    v_view = v_sb[:].rearrange("p j (h c) -> p j h c", c=HD + 1)
    # fill with ones via broadcast DMA; stage C overwrites the V columns,
    # leaving the per-head ones column (index HD) for the softmax denominator
    ones_dram = nc.inline_tensor(
        np.ones((NJS, H * (HD + 1)), ml_dtypes.bfloat16), "ones_fill"
    )
    nc.sync.dma_start(
        v_sb[:], ones_dram.ap()[None, :, :].to_broadcast((128, NJS, H * (HD + 1)))
    )
    avT = cpool.tile([128, NHP, SEQ], F32R, tag="avT")
    xT = cpool.tile([128, KSUB, SEQ], BF16, tag="xT")
    w_v_sb = cpool.tile([128, KSUB, VC], BF16, tag="wv")
    nc.sync.dma_start(w_v_sb[:], w_v_r[:])
    wproj_sb = cpool.tile([128, VC // 128, E], F32R, tag="wproj")
    nc.sync.dma_start(wproj_sb[:], w_proj_r[:].bitcast(F32R))
    bias_rep = cpool.tile([128, E], F32, tag="bias")
    nc.sync.dma_start(bias_rep[:], bias[None, :].to_broadcast((128, E)))

    out_r = out.rearrange("(p a) e -> p (a e)", p=128)

    # ---- Stage A: x^T via XBAR DMA transpose (bf16) ----
    for ksv in range(KSUB):
        nc.sync.dma_start_transpose(xT[:, ksv, :], x[:, ts(ksv, 128)])

    if stages <= 1:
        nc.sync.dma_start(
            out_r[:].bitcast(BF16)[:, 0 : KSUB * SEQ],
            xT[:].rearrange("p k s -> p (k s)"),
        )
        return

    # ---- Stage C: v = x @ w_v (natural layout) ----
    for jt in range(NJS):
        ps = psum.tile([128, VC], F32, tag="g")
        for ksv in range(KSUB):
            nc.tensor.matmul(
                ps[:],
                xT[:, ksv, ts(jt, 128)],
                w_v_sb[:, ksv, :],
                start=(ksv == 0),
                stop=(ksv == KSUB - 1),
            )
        nc.vector.tensor_copy(
            v_view[:, jt, :, 0:HD],
            ps[:].rearrange("p (h c) -> p h c", c=HD),
        )

    if stages <= 2:
        nc.sync.dma_start(
            out_r[:].bitcast(BF16)[:, 0 : NJS * H * (HD + 1)],
            v_sb[:].rearrange("p j c -> p (j c)"),
        )
        return

    # ---- per head-pair: B(hp) then D(hp) ----
    for hp in range(NHP):
        # B: q^T and k^T for this pair (c-tile hp -> q, hp+4 -> k)
        qTh = qkring.tile([128, SEQ], BF16, tag="qT", name=f"qT{hp}")
        kTh = qkring.tile([128, SEQ], BF16, tag="kT", name=f"kT{hp}")
        for ct, dest in ((hp, qTh), (hp + 4, kTh)):
            w_t = stream.tile([128, KSUB, 128], BF16, tag="wqk")
            nc.sync.dma_start(w_t[:], w_qk_r[:, :, ts(ct, 128)])
            for it in range(NIT):
                ps = psum.tile([128, ITILE], F32, tag="g")
                for ksv in range(KSUB):
                    nc.tensor.matmul(
                        ps[:],
                        w_t[:, ksv, :],
                        xT[:, ksv, ts(it, ITILE)],
                        start=(ksv == 0),
                        stop=(ksv == KSUB - 1),
                    )
                nc.vector.tensor_copy(dest[:, ts(it, ITILE)], ps[:])

        if stages <= 3:
            nc.sync.dma_start(
                out_r[:, ds(hp * 4096, SEQ)].bitcast(BF16)[:, 0:SEQ], qTh[:]
            )
            nc.sync.dma_start(
                out_r[:, ds(hp * 4096 + SEQ, SEQ)].bitcast(BF16)[:, 0:SEQ], kTh[:]
            )
            continue

        # D: attention for this pair (software-pipelined: AV lags one js so
        # the in-order PE queue never waits on the exp of the current js)
        for it in range(NIT):
            av_ps = [
                psum4.tile([HD + 1, ITILE], F32, tag="av", name=f"av{h01}")
                for h01 in range(2)
            ]

            def emit_av(js, es):
                for h01 in range(2):
                    nc.tensor.matmul(
                        av_ps[h01][:],
                        v_view[:, js, 2 * hp + h01, :],
                        es[h01][:],
                        start=(js == 0),
                        stop=(js == NJS - 1),
                    )

            pend = None
            for js in range(NJS):
                cur = []
                for h01 in range(2):
                    sl = slice(h01 * 64, h01 * 64 + 64)
                    sp = psum.tile([128, ITILE], F32, tag="s")
                    nc.tensor.matmul(
                        sp[:],
                        kTh[sl, ts(js, 128)],
                        qTh[sl, ts(it, ITILE)],
                        start=True,
                        stop=True,
                    )
                    e = epool.tile([128, ITILE], BF16, tag="e")
                    nc.scalar.activation(e[:], sp[:], AF.Exp)
                    cur.append(e)
                if pend is not None:
                    emit_av(js - 1, pend)
                pend = cur
            emit_av(NJS - 1, pend)

            for h01 in range(2):
                h = 2 * hp + h01
                rc = npool.tile([1, ITILE], F32, tag="rc")
                nc.vector.reciprocal(rc[:], av_ps[h01][HD : HD + 1, :])
                rr = npool.tile([64, ITILE], F32, tag="rr")
                nc.gpsimd.partition_broadcast(rr[:], rc[:])
                if h01 == 0:
                    nc.vector.tensor_mul(
                        avT[0:64, h // 2, ts(it, ITILE)],
                        av_ps[h01][0:HD, :],
                        rr[:],
                    )
                else:
                    tmp = npool.tile([64, ITILE], F32R, tag="tmp")
                    nc.vector.tensor_mul(tmp[:], av_ps[h01][0:HD, :], rr[:])
                    nc.sync.dma_start(avT[64:128, h // 2, ts(it, ITILE)], tmp[:])

    if stages <= 3:
        return
    if stages <= 4:
        nc.sync.dma_start(
            out_r[:, 0 : NHP * SEQ],
            avT[:].rearrange("p k s -> p (k s)").bitcast(F32),
        )
        return

    # ---- Stage E: out = avRow @ w_proj + bias (f32r) ----
    for it in range(SEQ // 128):
        for et in range(E // ITILE):
            ps = psum.tile([128, ITILE], F32, tag="s")
            for cs in range(VC // 128):
                nc.tensor.matmul(
                    ps[:],
                    avT[:, cs, ts(it, 128)],
                    wproj_sb[:, cs, ts(et, ITILE)],
                    start=(cs == 0),
                    stop=(cs == VC // 128 - 1),
                )
            o = opool.tile([128, ITILE], F32, tag="o")
            nc.vector.tensor_add(o[:], ps[:], bias_rep[:, ts(et, ITILE)])
            nc.sync.dma_start(out[ts(it, 128), ts(et, ITILE)], o[:])


# ---------------- host side ----------------

_CACHE = {}


def _get_runner():
    if "runner" not in _CACHE:
        import jax
        from jax.sharding import Mesh, PartitionSpec
        from jax.experimental.shard_map import shard_map
        from concourse import bass2jax

        nc = build_attention(iters=1)
        bass2jax.install_neuronx_cc_hook()

        in_names, out_names, out_avals, zero_shapes = [], [], [], []
        partition_name = nc.partition_id_tensor.name if nc.partition_id_tensor else None
        for alloc in nc.m.functions[0].allocations:
            if not isinstance(alloc, mybir.MemoryLocationSet):
                continue
            name = alloc.memorylocations[0].name
            if alloc.kind == "ExternalInput":
                if name != partition_name:
                    in_names.append(name)
            elif alloc.kind == "ExternalOutput":
                out_names.append(name)
                shape = tuple(alloc.tensor_shape)
                dtype = mybir.dt.np(alloc.dtype)
                out_avals.append(jax.core.ShapedArray(shape, dtype))
                zero_shapes.append((shape, dtype))
        n_params = len(in_names)
        n_outs = len(out_avals)
        all_names = in_names + out_names
        if partition_name is not None:
            all_names = all_names + [partition_name]
        donate = tuple(range(n_params, n_params + n_outs))

        def _body(*args):
            operands = list(args)
            if partition_name is not None:
                operands.append(bass2jax.partition_id_tensor())
            outs = bass2jax._bass_exec_p.bind(
                *operands,
                out_avals=tuple(out_avals),
                in_names=tuple(all_names),
                out_names=tuple(out_names),
                lowering_input_output_aliases=(),
                sim_require_finite=True,
                sim_require_nnan=True,
                nc=nc,
            )
            return tuple(outs)

        devices = jax.devices()[:8]
        mesh = Mesh(np.asarray(devices), ("core",))
        in_specs = (PartitionSpec("core"),) * (n_params + n_outs)
        out_specs = (PartitionSpec("core"),) * n_outs
        sharded = jax.jit(
            shard_map(
                _body,
                mesh=mesh,
                in_specs=in_specs,
                out_specs=out_specs,
                check_rep=False,
            ),
            donate_argnums=donate,
            keep_unused=True,
        )
        _CACHE["runner"] = (sharded, in_names, out_names, out_avals, zero_shapes)
    return _CACHE["runner"]


def _shard_inputs(x, w_qkv, w_proj, b_proj):
    """Per-core input dicts. Core c: batch c//2, head-group c%2."""
    SCALE = HD**-0.5
    bf16 = ml_dtypes.bfloat16
    in_maps = []
    zeros_bias = np.zeros_like(b_proj)
    for c in range(8):
        b = c // 2
        hg = c % 2
        qs = slice(hg * 512, (hg + 1) * 512)
        ks = slice(1024 + hg * 512, 1024 + (hg + 1) * 512)
        vs = slice(2048 + hg * 512, 2048 + (hg + 1) * 512)
        w_qk_c = np.concatenate(
            [w_qkv[:, qs] * np.float32(SCALE), w_qkv[:, ks]], axis=1
        ).astype(bf16)
        in_maps.append(
            {
                "x": x[b].astype(bf16),
                "w_qk": w_qk_c,
                "w_v": w_qkv[:, vs].astype(bf16),
                "w_proj": np.ascontiguousarray(w_proj[hg * 512 : (hg + 1) * 512]),
                "bias": b_proj if hg == 0 else zeros_bias,
            }
        )
    return in_maps


def kernel(x, w_qkv, w_proj, b_proj):
    import jax
    import jax.numpy as jnp

    x = np.asarray(x, dtype=np.float32)
    w_qkv = np.asarray(w_qkv, dtype=np.float32)
    w_proj = np.asarray(w_proj, dtype=np.float32)
    b_proj = np.asarray(b_proj, dtype=np.float32)

    sharded, in_names, out_names, out_avals, zero_shapes = _get_runner()
    in_maps = _shard_inputs(x, w_qkv, w_proj, b_proj)
    concat_in = [
        np.concatenate([in_maps[c][name] for c in range(8)], axis=0)
        for name in in_names
    ]
    zeros = [jnp.zeros((8 * s[0], *s[1:]), dt) for (s, dt) in zero_shapes]
    outs = sharded(*concat_in, *zeros)
    out_np = np.asarray(outs[out_names.index("out")]).reshape(8, SEQ, E)
    full = np.empty((4, SEQ, E), dtype=np.float32)
    for b in range(4):
        full[b] = out_np[2 * b] + out_np[2 * b + 1]
    return full
